# revision 14
# baseline (speedup 1.0000x reference)
"""ClusterNet (vq_codebook) kernel for 8x Trainium2 NeuronCores (Bass/Tile).

Reference math (ALPHA = 1):
    d2   = |z - c|^2                     z: (8192, 2048)  c: (512, 2048)
    Qun  = (1 + sqrt(d2))^-1
    Q    = Qun / rowsum(Qun)
    P    = (Q^2 / colsum(Q)) / rowsum(Q^2 / colsum(Q))
    out  = stack([Q, P])                 (2, 8192, 512) float32

Distribution: data-parallel over the batch — 1024 rows per core, centroids
replicated. The only cross-core communication is the global column-sum of Q
(512 floats), implemented as a direct peer-SBUF exchange: each core
remote-DMAs its [128,4]-laid-out partial colsum into slot (me XOR peer) of
every peer's mailbox (7 one-shot remote_dma_broadcast descriptors fired by a
single trigger after the kernel-entry barrier), then reduces the 8 slots
locally. This replaces the CC-core AllReduce (~40us of software latency)
with ~2KB*7 of D2D traffic.

Per-core pipeline (8 m-tiles of 128 rows):
  PE   : d2 accumulated in PSUM via bf16 matmuls (2048 contraction rows +
         4 affine rows carrying csq/zsq hi-lo splits); per-tile colsum of Q
         accumulated into a PSUM bank via ones-matvec; 1/s broadcast built
         with a rank-1 ones x rs_row matmul instead of a stride-0 DMA.
  ACT  : sqrt(d2) from PSUM; copy-with-accum emits rowsum(Qun); Square(Qun);
         P = W * (1/rowsum W) as copy-with-scale. All in one table set.
  DVE  : t = sim + 1; Qun = reciprocal_approx_fast(t); Q = Qun * (1/rowsum);
         W = q2 * (1/s) with fused row-accumulate (STT).
"""

import os
import sys

import numpy as np

if "/opt/trn_rl_repo" not in sys.path:
    sys.path.insert(0, "/opt/trn_rl_repo")

import ml_dtypes

import concourse.bacc as bacc
import concourse.mybir as mybir
import concourse.tile as tile
from concourse.bass_utils import run_bass_kernel_spmd

BF16 = ml_dtypes.bfloat16

N_CORES = 8
BS, NH, NC_CLUST = 8192, 2048, 512
B_CORE = BS // N_CORES          # 1024 rows per core
M_TILES = B_CORE // 128         # 8
K_TILES = NH // 128             # 16
KX = 4                          # affine rows: csq_hi, csq_lo, ones, ones

_nc_cache = None


def _build_nc():
    F = mybir.ActivationFunctionType
    A = mybir.AluOpType
    f32 = mybir.dt.float32
    bf16 = mybir.dt.bfloat16

    nc = bacc.Bacc("TRN2", target_bir_lowering=False, debug=False,
                   num_devices=N_CORES)
    zt = nc.dram_tensor("zt", [4, 128, 4 * B_CORE], bf16, kind="ExternalInput")
    ct = nc.dram_tensor("ct", [4, 128, 4 * NC_CLUST], bf16,
                        kind="ExternalInput")
    ztx_d = nc.dram_tensor("ztx", [KX, B_CORE], bf16, kind="ExternalInput")
    ctx_d = nc.dram_tensor("ctx", [KX, NC_CLUST], bf16, kind="ExternalInput")
    q_out = nc.dram_tensor("q", [B_CORE, NC_CLUST], f32, kind="ExternalOutput")
    p_out = nc.dram_tensor("p", [B_CORE, NC_CLUST], f32, kind="ExternalOutput")

    rsem = nc.alloc_semaphore("xch_rsem")
    lsem = nc.alloc_semaphore("xch_lsem")
    f32r = mybir.dt.float32r

    with tile.TileContext(nc) as tc:
        with (
            tc.tile_pool(name="zin", bufs=1) as zin,
            tc.tile_pool(name="cin", bufs=1) as cin,
            tc.tile_pool(name="work", bufs=1) as work,
            tc.tile_pool(name="small", bufs=1) as small,
            tc.tile_pool(name="pout", bufs=3) as pout,
            tc.tile_pool(name="psum", bufs=5, space="PSUM") as psum,
            tc.tile_pool(name="cpsum", bufs=1, space="PSUM") as cpsum,
            tc.tile_pool(name="bpsum", bufs=1, space="PSUM") as bpsum,
            tc.tile_pool(name="dram", bufs=1, space="DRAM") as dram,
        ):
            # input DMA: 4-k-tile chunks, partition-major DRAM layout so
            # each partition line is one 8KB/4KB contiguous descriptor
            zt_t, ct_t = [], []
            for g in range(4):
                ctg = cin.tile([128, 4, NC_CLUST], bf16, tag=f"ct{g}")
                nc.sync.dma_start(out=ctg, in_=ct.ap()[g].rearrange(
                    "p (j c) -> p j c", j=4))
                ztg = zin.tile([128, 4, B_CORE], bf16, tag=f"zt{g}")
                nc.sync.dma_start(out=ztg, in_=zt.ap()[g].rearrange(
                    "p (j b) -> p j b", j=4))
                for j in range(4):
                    ct_t.append(ctg[:, j, :])
                    zt_t.append(ztg[:, j, :])
            ztx = zin.tile([KX, B_CORE], bf16, tag="ztx")
            nc.sync.dma_start(out=ztx, in_=ztx_d.ap())
            ctx = cin.tile([KX, NC_CLUST], bf16, tag="ctx")
            nc.sync.dma_start(out=ctx, in_=ctx_d.ap())

            # workspaces
            sim_all = work.tile([128, M_TILES, NC_CLUST], f32, tag="sim")
            qun_all = work.tile([128, M_TILES, NC_CLUST], f32, tag="qun")
            q_all = work.tile([128, M_TILES, NC_CLUST], f32, tag="qa")
            q2_all = work.tile([128, M_TILES, NC_CLUST], f32, tag="q2")
            w_all = work.tile([128, M_TILES, NC_CLUST], f32, tag="w")
            qsum = small.tile([128, NC_CLUST], f32, tag="qsum")
            sq_all = small.tile([128, M_TILES], f32, tag="sq")
            rq_all = small.tile([128, M_TILES], f32, tag="rq")
            ws_all = small.tile([128, M_TILES], f32, tag="ws")
            rw_all = small.tile([128, M_TILES], f32, tag="rw")
            cs_sb = small.tile([1, NC_CLUST], f32, tag="cssb")
            rs_row = small.tile([1, NC_CLUST], bf16, tag="rsrow")
            ones_sb = small.tile([128, 1], f32, tag="ones")
            onesr_sb = small.tile([1, 128], bf16, tag="onesr")
            # mailbox: slot j receives the [128,4] colsum partial of core
            # (me XOR j); slot 0 is the local partial.
            mb = small.tile([128, 8, 4], f32, tag="mb")
            s4 = small.tile([128, 4], f32, tag="s4")
            rs4 = small.tile([128, 4], f32, tag="rs4")
            rs4_bf = small.tile([128, 4], bf16, tag="rs4bf")
            nc.vector.memset(ones_sb, 1.0)
            nc.vector.memset(onesr_sb, 1.0)
            nc.vector.memset(mb, 0.0)
            cs_dram = dram.tile([NC_CLUST], f32)
            rs_dram = dram.tile([NC_CLUST], bf16)

            # One-shot exchange descriptors (desc-gen only; fired by the
            # trigger after the entry barrier + local colsum are ready).
            # Instruction j targets XOR-partner j and lands in the peer's
            # mailbox slot j; slot==Dtpb keeps D2D slots (4-7) legal.
            for j in range(1, 8):
                rdests = [None] * 8
                rdests[j] = (0, j)
                nc.gpsimd.remote_dma_broadcast(
                    out_ap=mb[:, j, :], in_ap=mb[:, 0, :],
                    remote_sem=rsem, local_sem=lsem, rdests=rdests)

            # per-tile: matmuls -> sqrt -> Qun -> Q -> colsum(Q) in PSUM
            cps = cpsum.tile([1, NC_CLUST], f32, tag="cs")
            for m in range(M_TILES):
                ms = slice(m * 128, (m + 1) * 128)
                ps = psum.tile([128, NC_CLUST], f32, tag="mm")
                for k in range(K_TILES):
                    nc.tensor.matmul(ps, lhsT=zt_t[k][:, ms], rhs=ct_t[k],
                                     start=(k == 0), stop=False)
                nc.tensor.matmul(ps, lhsT=ztx[:, ms], rhs=ctx,
                                 start=False, stop=True)
                sim = sim_all[:, m, :]
                qun = qun_all[:, m, :]
                q = q_all[:, m, :]
                nc.scalar.activation(sim, ps, F.Sqrt)
                nc.vector.tensor_scalar_add(sim, sim, 1.0)      # in place
                nc.vector.reciprocal_approx_fast(out=qun, in_=sim)
                # rowsum(Qun) via copy-with-accum on ACT (sim is dead)
                nc.scalar.activation(sim, qun, F.Copy,
                                     accum_out=sq_all[:, m:m + 1])
                nc.scalar.activation(q2_all[:, m, :], qun, F.Square)
                nc.vector.reciprocal(rq_all[:, m:m + 1], sq_all[:, m:m + 1])
                nc.vector.tensor_scalar_mul(q, qun, rq_all[:, m:m + 1])
                nc.sync.dma_start(out=q_out.ap()[m * 128:(m + 1) * 128, :],
                                  in_=q)
                if m == 0:
                    nc.vector.tensor_copy(qsum, q)
                else:
                    nc.vector.tensor_add(qsum, qsum, q)

            # local colsum -> [128,4] mailbox slot 0 (via DRAM bounce)
            nc.tensor.matmul(cps, lhsT=ones_sb, rhs=qsum,
                             start=True, stop=True)
            nc.vector.tensor_copy(cs_sb, cps)
            nc.sync.dma_start(out=cs_dram[:], in_=cs_sb)
            nc.sync.dma_start(
                out=mb[:, 0, :],
                in_=cs_dram[:].rearrange("(p c) -> p c", p=128))

            # fire the exchange once every peer entered the kernel.
            # The waits are emitted with threshold 0 (satisfiable in the
            # Tile scheduling sim, which cannot see cross-core increments)
            # and patched to their real values after scheduling, before
            # nc.compile().
            assert nc._bir_kernel_barrier_sem is not None
            bw = nc.gpsimd.wait_ge(nc._bir_kernel_barrier_sem, 0)
            trig = nc.gpsimd.trigger_dma(count=None)
            tile.add_dep_helper(trig.ins, bw.ins, sync=False,
                                reason="exchange gated by entry barrier")

            # wait for all 7 peers' partials (2 sem incs per arrival)
            rw = nc.vector.wait_ge(rsem, 0)
            red = nc.vector.reduce_sum(
                s4, mb.rearrange("p j c -> p c j"),
                axis=mybir.AxisListType.X)
            tile.add_dep_helper(red.ins, rw.ins, sync=False,
                                reason="mailbox gated by remote sem")
            nc.vector.reciprocal(rs4, s4)
            nc.vector.tensor_copy(rs4_bf, rs4)
            nc.sync.dma_start(out=rs_dram[:], in_=rs4_bf)
            nc.sync.dma_start(
                out=rs_row,
                in_=rs_dram[:].rearrange("(o c) -> o c", o=1))
            rs_bc = bpsum.tile([128, NC_CLUST], f32, tag="rsbc")
            nc.tensor.matmul(rs_bc, lhsT=onesr_sb, rhs=rs_row,
                             start=True, stop=True, skip_group_check=True)

            # P phase: W = q2 * (1/s) with fused rowsum; P = W * (1/ws)
            for m in range(M_TILES):
                nc.vector.scalar_tensor_tensor(
                    out=w_all[:, m, :], in0=q2_all[:, m, :],
                    scalar=0.0, in1=rs_bc,
                    op0=A.bypass, op1=A.mult,
                    accum_out=ws_all[:, m:m + 1])
                nc.vector.reciprocal(rw_all[:, m:m + 1], ws_all[:, m:m + 1])
                pt = pout.tile([128, NC_CLUST], f32, tag="pt")
                nc.scalar.activation(pt, w_all[:, m, :], F.Copy,
                                     scale=rw_all[:, m:m + 1])
                nc.sync.dma_start(out=p_out.ap()[m * 128:(m + 1) * 128, :],
                                  in_=pt)

    # Patch the placeholder waits to their real thresholds now that the
    # Tile scheduling sim has run, and register the replica group so
    # compile() inserts the kernel-entry barrier AllGather prelude.
    nc._bir_kernel_barrier_sem_replica_groups.append(set(range(N_CORES)))
    for w, val in ((bw, nc.bir_kernel_barrier_sem_inc), (rw, 14)):
        sw = [x for x in w.ins.sync_info.on_wait if x.sync_type == "semaphore"]
        assert sw, f"placeholder wait lost its semaphore: {w.ins}"
        sw[0].wait_value = val
    nc.compile()

    # The exchange is only safe if both patched waits survived compilation.
    want = {(rsem.num, 14), (nc._bir_kernel_barrier_sem.num,
                             nc.bir_kernel_barrier_sem_inc)}
    seen = set()
    for b in nc.m.functions[0].blocks:
        for i in b.instructions:
            si = i.sync_info
            if si is None:
                continue
            for x in si.on_wait:
                if x.sync_type == "semaphore":
                    seen.add((x.id, x.wait_value))
    missing = want - seen
    assert not missing, f"patched waits missing from compiled module: {missing}"
    return nc


def _get_nc():
    global _nc_cache
    if _nc_cache is None:
        _nc_cache = _build_nc()
    return _nc_cache


def _split_hi_lo(x64):
    """Split float64 values into bf16 hi + bf16 lo with hi + lo ~= x."""
    hi = x64.astype(BF16)
    lo = (x64 - hi.astype(np.float64)).astype(BF16)
    return hi, lo


def _prep_inputs(z, centroids):
    z = np.asarray(z, dtype=np.float32)
    c = np.asarray(centroids, dtype=np.float32)

    csq = np.sum(c.astype(np.float64) ** 2, axis=1)      # (512,)
    csq_hi, csq_lo = _split_hi_lo(csq)
    ctx = np.empty((KX, NC_CLUST), dtype=BF16)
    ctx[0] = csq_hi
    ctx[1] = csq_lo
    ctx[2] = BF16(1.0)
    ctx[3] = BF16(1.0)

    zsq = np.sum(z.astype(np.float64) ** 2, axis=1)      # (8192,)
    zsq_hi, zsq_lo = _split_hi_lo(zsq)

    # [g, p, j, b]: contraction row h = (4 g + j) * 128 + p
    zT_bf = z.T.reshape(4, 4, 128, BS).transpose(0, 2, 1, 3).astype(BF16)
    ct_full = np.ascontiguousarray(
        (-2.0 * c.T).reshape(4, 4, 128, NC_CLUST).transpose(0, 2, 1, 3)
    ).astype(BF16).reshape(4, 128, 4 * NC_CLUST)

    in_maps = []
    for core in range(N_CORES):
        s = slice(core * B_CORE, (core + 1) * B_CORE)
        ztx = np.empty((KX, B_CORE), dtype=BF16)
        ztx[0] = BF16(1.0)
        ztx[1] = BF16(1.0)
        ztx[2] = zsq_hi[s]
        ztx[3] = zsq_lo[s]
        zt_core = np.ascontiguousarray(
            zT_bf[:, :, :, s]).reshape(4, 128, 4 * B_CORE)
        in_maps.append({"zt": zt_core, "ct": ct_full,
                        "ztx": ztx, "ctx": ctx})
    return in_maps


def run(z, centroids, trace=False, trace_cores=None):
    """Run on the 8 NeuronCores. Returns (out, BassKernelResults)."""
    nc = _get_nc()
    in_maps = _prep_inputs(z, centroids)
    res = run_bass_kernel_spmd(
        nc, in_maps, list(range(N_CORES)),
        trace=trace, trace_cores=trace_cores,
    )
    q = np.concatenate([res.results[c]["q"] for c in range(N_CORES)], axis=0)
    p = np.concatenate([res.results[c]["p"] for c in range(N_CORES)], axis=0)
    out = np.stack([q, p]).astype(np.float32)
    return out, res


def kernel(z, centroids):
    out, _ = run(z, centroids)
    return out


# revision 16
# speedup vs baseline: 1.0148x; 1.0148x over previous
"""ClusterNet (vq_codebook) kernel for 8x Trainium2 NeuronCores (Bass/Tile).

Reference math (ALPHA = 1):
    d2   = |z - c|^2                     z: (8192, 2048)  c: (512, 2048)
    Qun  = (1 + sqrt(d2))^-1
    Q    = Qun / rowsum(Qun)
    P    = (Q^2 / colsum(Q)) / rowsum(Q^2 / colsum(Q))
    out  = stack([Q, P])                 (2, 8192, 512) float32

Distribution: data-parallel over the batch — 1024 rows per core, centroids
replicated. The only cross-core communication is the global column-sum of Q
(512 floats), implemented as a direct peer-SBUF exchange: each core
remote-DMAs its [128,4]-laid-out partial colsum into slot (me XOR peer) of
every peer's mailbox (7 one-shot remote_dma_broadcast descriptors fired by a
single trigger after the kernel-entry barrier), then reduces the 8 slots
locally. This replaces the CC-core AllReduce (~40us of software latency)
with ~2KB*7 of D2D traffic.

Per-core pipeline (8 m-tiles of 128 rows):
  PE   : d2 accumulated in PSUM via bf16 matmuls (2048 contraction rows +
         4 affine rows carrying csq/zsq hi-lo splits); per-tile colsum of Q
         accumulated into a PSUM bank via ones-matvec; 1/s broadcast built
         with a rank-1 ones x rs_row matmul instead of a stride-0 DMA.
  ACT  : sqrt(d2) from PSUM; copy-with-accum emits rowsum(Qun); Square(Qun);
         P = W * (1/rowsum W) as copy-with-scale. All in one table set.
  DVE  : t = sim + 1; Qun = reciprocal_approx_fast(t); Q = Qun * (1/rowsum);
         W = q2 * (1/s) with fused row-accumulate (STT).
"""

import os
import sys

import numpy as np

if "/opt/trn_rl_repo" not in sys.path:
    sys.path.insert(0, "/opt/trn_rl_repo")

import ml_dtypes

import concourse.bacc as bacc
import concourse.mybir as mybir
import concourse.tile as tile
from concourse.bass_utils import run_bass_kernel_spmd

BF16 = ml_dtypes.bfloat16

N_CORES = 8
BS, NH, NC_CLUST = 8192, 2048, 512
B_CORE = BS // N_CORES          # 1024 rows per core
M_TILES = B_CORE // 128         # 8
K_TILES = NH // 128             # 16
KX = 4                          # affine rows: csq_hi, csq_lo, ones, ones

_nc_cache = None


def _build_nc():
    F = mybir.ActivationFunctionType
    A = mybir.AluOpType
    f32 = mybir.dt.float32
    bf16 = mybir.dt.bfloat16

    nc = bacc.Bacc("TRN2", target_bir_lowering=False, debug=False,
                   num_devices=N_CORES)
    zt = nc.dram_tensor("zt", [4, 128, 4 * B_CORE], bf16, kind="ExternalInput")
    ct = nc.dram_tensor("ct", [4, 128, 4 * NC_CLUST], bf16,
                        kind="ExternalInput")
    ztx_d = nc.dram_tensor("ztx", [KX, B_CORE], bf16, kind="ExternalInput")
    ctx_d = nc.dram_tensor("ctx", [KX, NC_CLUST], bf16, kind="ExternalInput")
    q_out = nc.dram_tensor("q", [B_CORE, NC_CLUST], f32, kind="ExternalOutput")
    p_out = nc.dram_tensor("p", [B_CORE, NC_CLUST], f32, kind="ExternalOutput")

    rsem = nc.alloc_semaphore("xch_rsem")
    lsem = nc.alloc_semaphore("xch_lsem")
    f32r = mybir.dt.float32r

    with tile.TileContext(nc) as tc:
        with (
            tc.tile_pool(name="zin", bufs=1) as zin,
            tc.tile_pool(name="cin", bufs=1) as cin,
            tc.tile_pool(name="work", bufs=1) as work,
            tc.tile_pool(name="small", bufs=1) as small,
            tc.tile_pool(name="pout", bufs=3) as pout,
            tc.tile_pool(name="psum", bufs=5, space="PSUM") as psum,
            tc.tile_pool(name="cpsum", bufs=1, space="PSUM") as cpsum,
            tc.tile_pool(name="bpsum", bufs=1, space="PSUM") as bpsum,
            tc.tile_pool(name="dram", bufs=1, space="DRAM") as dram,
        ):
            # input DMA: 4-k-tile chunks, partition-major DRAM layout so
            # each partition line is one 8KB/4KB contiguous descriptor
            zt_t, ct_t = [], []
            for g in range(4):
                ctg = cin.tile([128, 4, NC_CLUST], bf16, tag=f"ct{g}")
                nc.sync.dma_start(out=ctg, in_=ct.ap()[g].rearrange(
                    "p (j c) -> p j c", j=4))
                ztg = zin.tile([128, 4, B_CORE], bf16, tag=f"zt{g}")
                nc.sync.dma_start(out=ztg, in_=zt.ap()[g].rearrange(
                    "p (j b) -> p j b", j=4))
                for j in range(4):
                    ct_t.append(ctg[:, j, :])
                    zt_t.append(ztg[:, j, :])
            ztx = zin.tile([KX, B_CORE], bf16, tag="ztx")
            nc.sync.dma_start(out=ztx, in_=ztx_d.ap())
            ctx = cin.tile([KX, NC_CLUST], bf16, tag="ctx")
            nc.sync.dma_start(out=ctx, in_=ctx_d.ap())

            # workspaces
            sim_all = work.tile([128, M_TILES, NC_CLUST], f32, tag="sim")
            qun_all = work.tile([128, M_TILES, NC_CLUST], f32, tag="qun")
            q_all = work.tile([128, M_TILES, NC_CLUST], f32, tag="qa")
            q2_all = work.tile([128, M_TILES, NC_CLUST], f32, tag="q2")
            w_all = work.tile([128, M_TILES, NC_CLUST], f32, tag="w")
            qsum = small.tile([128, NC_CLUST], f32, tag="qsum")
            sq_all = small.tile([128, M_TILES], f32, tag="sq")
            rq_all = small.tile([128, M_TILES], f32, tag="rq")
            ws_all = small.tile([128, M_TILES], f32, tag="ws")
            rw_all = small.tile([128, M_TILES], f32, tag="rw")
            cs_sb = small.tile([1, NC_CLUST], f32, tag="cssb")
            rs_row = small.tile([1, NC_CLUST], bf16, tag="rsrow")
            ones_sb = small.tile([128, 1], f32, tag="ones")
            onesr_sb = small.tile([1, 128], bf16, tag="onesr")
            # mailbox: slot j receives the [128,4] colsum partial of core
            # (me XOR j); slot 0 is the local partial.
            mb = small.tile([128, 8, 4], f32, tag="mb")
            s4 = small.tile([128, 4], f32, tag="s4")
            rs4 = small.tile([128, 4], f32, tag="rs4")
            rs4_bf = small.tile([128, 4], bf16, tag="rs4bf")
            nc.vector.memset(ones_sb, 1.0)
            nc.vector.memset(onesr_sb, 1.0)
            nc.vector.memset(mb, 0.0)
            cs_dram = dram.tile([NC_CLUST], f32)
            rs_dram = dram.tile([NC_CLUST], bf16)

            # One-shot exchange descriptors (desc-gen only; fired by the
            # trigger after the entry barrier + local colsum are ready).
            # Instruction j targets XOR-partner j and lands in the peer's
            # mailbox slot j; slot==Dtpb keeps D2D slots (4-7) legal.
            preps = []
            for j in range(1, 8):
                rdests = [None] * 8
                rdests[j] = (0, j)
                preps.append(nc.gpsimd.remote_dma_broadcast(
                    out_ap=mb[:, j, :], in_ap=mb[:, 0, :],
                    remote_sem=rsem, local_sem=lsem, rdests=rdests))

            # per-tile: matmuls -> sqrt -> Qun -> Q -> colsum(Q) in PSUM
            cps = cpsum.tile([1, NC_CLUST], f32, tag="cs")
            for m in range(M_TILES):
                ms = slice(m * 128, (m + 1) * 128)
                ps = psum.tile([128, NC_CLUST], f32, tag="mm")
                for k in range(K_TILES):
                    nc.tensor.matmul(ps, lhsT=zt_t[k][:, ms], rhs=ct_t[k],
                                     start=(k == 0), stop=False)
                nc.tensor.matmul(ps, lhsT=ztx[:, ms], rhs=ctx,
                                 start=False, stop=True)
                sim = sim_all[:, m, :]
                qun = qun_all[:, m, :]
                q = q_all[:, m, :]
                nc.scalar.activation(sim, ps, F.Sqrt)
                nc.vector.tensor_scalar_add(sim, sim, 1.0)      # in place
                nc.vector.reciprocal_approx_fast(out=qun, in_=sim)
                # rowsum(Qun) via copy-with-accum on ACT (sim is dead)
                nc.scalar.activation(sim, qun, F.Copy,
                                     accum_out=sq_all[:, m:m + 1])
                nc.scalar.activation(q2_all[:, m, :], qun, F.Square)
                nc.vector.reciprocal(rq_all[:, m:m + 1], sq_all[:, m:m + 1])
                nc.vector.tensor_scalar_mul(q, qun, rq_all[:, m:m + 1])
                nc.sync.dma_start(out=q_out.ap()[m * 128:(m + 1) * 128, :],
                                  in_=q)
                if m == 0:
                    nc.vector.tensor_copy(qsum, q)
                else:
                    nc.vector.tensor_add(qsum, qsum, q)

            # local colsum -> [128,4] mailbox slot 0 (via DRAM bounce)
            nc.tensor.matmul(cps, lhsT=ones_sb, rhs=qsum,
                             start=True, stop=True)
            nc.vector.tensor_copy(cs_sb, cps)
            nc.sync.dma_start(out=cs_dram[:], in_=cs_sb)
            nc.sync.dma_start(
                out=mb[:, 0, :],
                in_=cs_dram[:].rearrange("(p c) -> p c", p=128))

            # fire the exchange once every peer entered the kernel.
            # The waits are emitted with threshold 0 (satisfiable in the
            # Tile scheduling sim, which cannot see cross-core increments)
            # and patched to their real values after scheduling, before
            # nc.compile().
            assert nc._bir_kernel_barrier_sem is not None
            bw = nc.gpsimd.wait_ge(nc._bir_kernel_barrier_sem, 0)
            trig = nc.gpsimd.trigger_dma(count=None)
            tile.add_dep_helper(trig.ins, bw.ins, sync=False,
                                reason="exchange gated by entry barrier")
            for p in preps:
                # scheduler must keep desc-gen before the barrier wait (so
                # it hides under the matmul phase) and before the trigger
                # (an untriggered desc is only flushed by the final drain).
                tile.add_dep_helper(bw.ins, p.ins, sync=False,
                                    reason="prep desc-gen before barrier")
                tile.add_dep_helper(trig.ins, p.ins, sync=False,
                                    reason="trigger after all preps")

            # wait for all 7 peers' partials (2 sem incs per arrival)
            rw = nc.vector.wait_ge(rsem, 0)
            red = nc.vector.reduce_sum(
                s4, mb.rearrange("p j c -> p c j"),
                axis=mybir.AxisListType.X)
            tile.add_dep_helper(red.ins, rw.ins, sync=False,
                                reason="mailbox gated by remote sem")
            nc.vector.reciprocal(rs4, s4)
            nc.vector.tensor_copy(rs4_bf, rs4)
            nc.sync.dma_start(out=rs_dram[:], in_=rs4_bf)
            nc.sync.dma_start(
                out=rs_row,
                in_=rs_dram[:].rearrange("(o c) -> o c", o=1))
            rs_bc = bpsum.tile([128, NC_CLUST], f32, tag="rsbc")
            nc.tensor.matmul(rs_bc, lhsT=onesr_sb, rhs=rs_row,
                             start=True, stop=True, skip_group_check=True)

            # P phase: W = q2 * (1/s) with fused rowsum; P = W * (1/ws)
            for m in range(M_TILES):
                nc.vector.scalar_tensor_tensor(
                    out=w_all[:, m, :], in0=q2_all[:, m, :],
                    scalar=0.0, in1=rs_bc,
                    op0=A.bypass, op1=A.mult,
                    accum_out=ws_all[:, m:m + 1])
                nc.vector.reciprocal(rw_all[:, m:m + 1], ws_all[:, m:m + 1])
                pt = pout.tile([128, NC_CLUST], f32, tag="pt")
                nc.scalar.activation(pt, w_all[:, m, :], F.Copy,
                                     scale=rw_all[:, m:m + 1])
                nc.sync.dma_start(out=p_out.ap()[m * 128:(m + 1) * 128, :],
                                  in_=pt)

    # Patch the placeholder waits to their real thresholds now that the
    # Tile scheduling sim has run, and register the replica group so
    # compile() inserts the kernel-entry barrier AllGather prelude.
    nc._bir_kernel_barrier_sem_replica_groups.append(set(range(N_CORES)))
    for w, val in ((bw, nc.bir_kernel_barrier_sem_inc), (rw, 14)):
        sw = [x for x in w.ins.sync_info.on_wait if x.sync_type == "semaphore"]
        assert sw, f"placeholder wait lost its semaphore: {w.ins}"
        sw[0].wait_value = val
    nc.compile()

    # The exchange is only safe if both patched waits survived compilation.
    want = {(rsem.num, 14), (nc._bir_kernel_barrier_sem.num,
                             nc.bir_kernel_barrier_sem_inc)}
    seen = set()
    for b in nc.m.functions[0].blocks:
        for i in b.instructions:
            si = i.sync_info
            if si is None:
                continue
            for x in si.on_wait:
                if x.sync_type == "semaphore":
                    seen.add((x.id, x.wait_value))
    missing = want - seen
    assert not missing, f"patched waits missing from compiled module: {missing}"
    return nc


def _get_nc():
    global _nc_cache
    if _nc_cache is None:
        _nc_cache = _build_nc()
    return _nc_cache


def _split_hi_lo(x64):
    """Split float64 values into bf16 hi + bf16 lo with hi + lo ~= x."""
    hi = x64.astype(BF16)
    lo = (x64 - hi.astype(np.float64)).astype(BF16)
    return hi, lo


def _prep_inputs(z, centroids):
    z = np.asarray(z, dtype=np.float32)
    c = np.asarray(centroids, dtype=np.float32)

    csq = np.sum(c.astype(np.float64) ** 2, axis=1)      # (512,)
    csq_hi, csq_lo = _split_hi_lo(csq)
    ctx = np.empty((KX, NC_CLUST), dtype=BF16)
    ctx[0] = csq_hi
    ctx[1] = csq_lo
    ctx[2] = BF16(1.0)
    ctx[3] = BF16(1.0)

    zsq = np.sum(z.astype(np.float64) ** 2, axis=1)      # (8192,)
    zsq_hi, zsq_lo = _split_hi_lo(zsq)

    # [g, p, j, b]: contraction row h = (4 g + j) * 128 + p
    zT_bf = z.T.reshape(4, 4, 128, BS).transpose(0, 2, 1, 3).astype(BF16)
    ct_full = np.ascontiguousarray(
        (-2.0 * c.T).reshape(4, 4, 128, NC_CLUST).transpose(0, 2, 1, 3)
    ).astype(BF16).reshape(4, 128, 4 * NC_CLUST)

    in_maps = []
    for core in range(N_CORES):
        s = slice(core * B_CORE, (core + 1) * B_CORE)
        ztx = np.empty((KX, B_CORE), dtype=BF16)
        ztx[0] = BF16(1.0)
        ztx[1] = BF16(1.0)
        ztx[2] = zsq_hi[s]
        ztx[3] = zsq_lo[s]
        zt_core = np.ascontiguousarray(
            zT_bf[:, :, :, s]).reshape(4, 128, 4 * B_CORE)
        in_maps.append({"zt": zt_core, "ct": ct_full,
                        "ztx": ztx, "ctx": ctx})
    return in_maps


def run(z, centroids, trace=False, trace_cores=None):
    """Run on the 8 NeuronCores. Returns (out, BassKernelResults)."""
    nc = _get_nc()
    in_maps = _prep_inputs(z, centroids)
    res = run_bass_kernel_spmd(
        nc, in_maps, list(range(N_CORES)),
        trace=trace, trace_cores=trace_cores,
    )
    q = np.concatenate([res.results[c]["q"] for c in range(N_CORES)], axis=0)
    p = np.concatenate([res.results[c]["p"] for c in range(N_CORES)], axis=0)
    out = np.stack([q, p]).astype(np.float32)
    return out, res


def kernel(z, centroids):
    out, _ = run(z, centroids)
    return out
